# revision 13
# baseline (speedup 1.0000x reference)
"""Trainium2 Bass kernel for a small GPT (multi-head attention with
multiplicative masks, applied NM times per layer, + MLP, + vocab head).

Sharding over 8 NeuronCores (SPMD, zero collectives):
  core c -> batch element b = c // 2   (each batch element's transformer body
            is computed redundantly on a pair of cores),
            vocab shard     h = c % 2  (the LM head weight is split in two
            along the vocab dim; each core of the pair emits logits for its
            half of the (padded) vocabulary for all T tokens of its batch
            element).
The per-core program is identical; only input data differs (SPMD).

All matmuls run in bf16 with fp32 PSUM accumulation; the residual stream,
layernorm statistics and softmax denominators stay in fp32.

Internal layouts (SBUF, partition dim first, P=128):
  x      [P, TN, D]  fp32   token-partition residual stream, t = tn*P + tp
  hT     [P, DN, T]  bf16   LN output, transposed: hT[dp, dn, t] = h[t, dn*P+dp]
  QT/KT  [P, DN, T]  bf16   q/k transposed; head h lives on partitions
                            (h%2)*64..(h%2)*64+64 at dn = h//2
  Vaug   [P, TN, H, DH+1] bf16  v in token layout + ones column (col DH) so the
                            ctx matmul also produces softmax denominators
  expT   [P, T]      bf16   exp(mask * scores^T) for one tk-tile (streamed)
  ctxTu  [64, H, T]  bf16   unnormalized ctx^T per head
  ctxn   [P, TN, D]  bf16   normalized ctx, token layout (after transpose T1)
  ctxTn  [P, DN, T]  bf16   normalized ctx^T (after transpose T2) for out-proj
"""

import math
from contextlib import ExitStack
from dataclasses import dataclass

import numpy as np
import ml_dtypes

import concourse.bass as bass
import concourse.mybir as mybir
import concourse.tile as tile
from concourse import bacc
from concourse.masks import make_identity

F32 = mybir.dt.float32
BF16 = mybir.dt.bfloat16
I32 = mybir.dt.int32
AF = mybir.ActivationFunctionType
ALU = mybir.AluOpType
P = 128

# ---------------- model dims (from the reference problem) ----------------
B_FULL, T_FULL, D_FULL, H_FULL, L_FULL = 4, 1024, 1024, 16, 6
V_FULL, NM_FULL, DFF_FULL = 50257, 2, 4 * 1024
VS_FULL = 25600  # per-core padded vocab shard (2*25600 = 51200 >= 50257)
N_CORES = 8


@dataclass(frozen=True)
class Cfg:
    T: int = T_FULL
    D: int = D_FULL
    H: int = H_FULL
    DH: int = 64
    L: int = L_FULL
    NM: int = NM_FULL
    DFF: int = DFF_FULL
    V: int = V_FULL
    VS: int = VS_FULL
    eps: float = 1e-5
    debug_taps: tuple = ()

    @property
    def TN(self):
        return self.T // P

    @property
    def DN(self):
        return self.D // P

    @property
    def FFN(self):
        return self.DFF // P

    tqc0: int = 512

    @property
    def TQC(self):  # tq/free-dim chunk size for matmul N (psum bank = 512 f32)
        return min(self.tqc0, self.T)

    @property
    def NJ(self):
        return self.T // self.TQC

    @property
    def HPB(self):  # heads per 128-partition block
        return P // self.DH


def _chunks(total, w):
    return [(s, min(w, total - s)) for s in range(0, total, w)]


class GPTBuilder:
    def __init__(self, cfg: Cfg):
        self.cfg = cfg
        self.nc = bacc.Bacc("TRN2", target_bir_lowering=False, debug=False)
        self.taps = {}

    # ---------------- dram params ----------------
    def declare_params(self):
        nc, cfg = self.nc, self.cfg
        dt = nc.dram_tensor
        self.d_idx = dt("idx", [cfg.T, 1], I32, kind="ExternalInput")
        self.d_tok = dt("tok_emb", [cfg.V, cfg.D], F32, kind="ExternalInput")
        self.d_pos = dt("pos_r", [P, cfg.TN, cfg.D], F32, kind="ExternalInput")
        self.d_maskT = dt("masksT_r", [cfg.NM, P, cfg.TN, cfg.T], BF16,
                          kind="ExternalInput")
        self.d_wq = dt("wq_r", [cfg.L, P, cfg.DN, cfg.D], BF16, kind="ExternalInput")
        self.d_wk = dt("wk_r", [cfg.L, P, cfg.DN, cfg.D], BF16, kind="ExternalInput")
        self.d_wv = dt("wv_r", [cfg.L, P, cfg.DN, cfg.D], BF16, kind="ExternalInput")
        self.d_wo = dt("wo_r", [cfg.L, P, cfg.DN, cfg.D], BF16, kind="ExternalInput")
        self.d_w1 = dt("w1_r", [cfg.L, P, cfg.DN, cfg.DFF], BF16, kind="ExternalInput")
        self.d_w2 = dt("w2_r", [cfg.L, P, cfg.FFN, cfg.D], BF16, kind="ExternalInput")
        self.d_ln1w = dt("ln1w_r", [cfg.L, P, cfg.DN], F32, kind="ExternalInput")
        self.d_ln1b = dt("ln1b_r", [cfg.L, P, cfg.DN], F32, kind="ExternalInput")
        self.d_ln2w = dt("ln2w_r", [cfg.L, P, cfg.DN], F32, kind="ExternalInput")
        self.d_ln2b = dt("ln2b_r", [cfg.L, P, cfg.DN], F32, kind="ExternalInput")
        self.d_bq = dt("bq_r", [cfg.L, P, cfg.DN], F32, kind="ExternalInput")
        self.d_bk = dt("bk_r", [cfg.L, P, cfg.DN], F32, kind="ExternalInput")
        self.d_b1 = dt("b1_r", [cfg.L, P, cfg.FFN], F32, kind="ExternalInput")
        self.d_bvrow = dt("bv_row", [cfg.L, 1, cfg.D], BF16, kind="ExternalInput")
        self.d_borow = dt("bo_row", [cfg.L, 1, cfg.D], BF16, kind="ExternalInput")
        self.d_b2row = dt("b2_row", [cfg.L, 1, cfg.D], BF16, kind="ExternalInput")
        self.d_lnfw = dt("lnfw_r", [P, cfg.DN], F32, kind="ExternalInput")
        self.d_lnfb = dt("lnfb_r", [P, cfg.DN], F32, kind="ExternalInput")
        self.d_head = dt("head_r", [P, cfg.DN, cfg.VS], BF16, kind="ExternalInput")
        self.d_out = dt("out", [cfg.T, cfg.VS], F32, kind="ExternalOutput")

    def tap(self, name, ap, dtype=None):
        """Optionally expose an SBUF tile as an extra output (debug)."""
        if name not in self.cfg.debug_taps or name in self.taps:
            return
        nc = self.nc
        dt = dtype or ap.dtype
        d = nc.dram_tensor(f"tap_{name}", list(ap.shape), dt, kind="ExternalOutput")
        nc.sync.dma_start(out=d[:], in_=ap)
        self.taps[name] = d

    # ---------------- pools ----------------
    def open_pools(self, ctx: ExitStack):
        tc = self.tc
        self.p1 = ctx.enter_context(tc.tile_pool(name="p1", bufs=1))
        self.p2 = ctx.enter_context(tc.tile_pool(name="p2", bufs=2))
        self.p3 = ctx.enter_context(tc.tile_pool(name="p3", bufs=2))
        self.ps_mm = ctx.enter_context(tc.tile_pool(name="ps_mm", bufs=3, space="PSUM"))
        self.ps_ctx = ctx.enter_context(tc.tile_pool(name="ps_ctx", bufs=2, space="PSUM"))
        self.ps_tr = ctx.enter_context(tc.tile_pool(name="ps_tr", bufs=2, space="PSUM"))
        self.ps_den = ctx.enter_context(tc.tile_pool(name="ps_den", bufs=1, space="PSUM"))

    # ---------------- building blocks ----------------
    def emit_constants(self):
        nc, cfg = self.nc, self.cfg
        self.identF = self.p1.tile([P, P], F32, tag="identF")
        make_identity(nc, self.identF[:])
        self.identB = self.p1.tile([P, P], BF16, tag="identB")
        nc.vector.tensor_copy(out=self.identB[:], in_=self.identF[:])
        self.onesB = self.p1.tile([P, 1], BF16, tag="onesB")
        nc.vector.memset(self.onesB[:], 1.0)
        self.ones_row = self.p1.tile([1, P], BF16, tag="ones_row")
        nc.vector.memset(self.ones_row[:], 1.0)
        self.epsA = self.p1.tile([P, 1], F32, tag="epsA")
        nc.vector.memset(self.epsA[:], cfg.eps)
        self.onesF = self.p1.tile([P, 1], F32, tag="onesF")
        nc.vector.memset(self.onesF[:], 1.0)

    def emit_embedding(self):
        nc, cfg = self.nc, self.cfg
        self.x = self.p1.tile([P, cfg.TN, cfg.D], F32, tag="x")
        pos = self.p1.tile([P, cfg.TN, cfg.D], F32, tag="big32")
        nc.sync.dma_start(out=pos[:], in_=self.d_pos[:])
        for tn in range(cfg.TN):
            it = self.p2.tile([P, 1], I32, tag="idx")
            nc.sync.dma_start(out=it[:], in_=self.d_idx[tn * P:(tn + 1) * P, :])
            nc.gpsimd.indirect_dma_start(
                out=self.x[:, tn, :], out_offset=None,
                in_=self.d_tok[:],
                in_offset=bass.IndirectOffsetOnAxis(ap=it[:, :1], axis=0),
            )
        nc.vector.tensor_add(out=self.x[:], in0=self.x[:], in1=pos[:])
        self.tap("x0", self.x[:])

    def emit_ln_to_hT(self, w_col, b_col, tag_out="hT", tapname=None):
        """LayerNorm(x) -> transposed hT [P, DN, T] bf16.

        w_col/b_col: [P, DN] fp32 SBUF tiles (per-d scale/bias, d on partitions
        in the transposed layout, so they fold into the PSUM evacuation).
        """
        nc, cfg = self.nc, self.cfg
        TN, DN, D = cfg.TN, cfg.DN, cfg.D
        # --- stats: bn_stats/bn_aggr per token tile -> mean, var ---
        ngrp = max(1, D // 512)
        gsz = D // ngrp
        mv = self.p1.tile([P, TN, 2], F32, tag="mv")
        for tn in range(TN):
            bnst = self.p2.tile([P, ngrp, 6], F32, tag="bnst")
            for g in range(ngrp):
                nc.vector.bn_stats(out=bnst[:, g, :],
                                   in_=self.x[:, tn, g * gsz:(g + 1) * gsz])
            nc.vector.bn_aggr(out=mv[:, tn, :], in_=bnst[:])
        negmean = self.p1.tile([P, TN], F32, tag="negmean")
        rstd = self.p1.tile([P, TN], F32, tag="rstd")
        std = self.p1.tile([P, TN], F32, tag="std")
        nc.vector.tensor_scalar_mul(negmean[:], mv[:, :, 0], -1.0)
        # std = sqrt(var + eps); rstd = 1/std  (accurate reciprocal on DVE)
        nc.scalar.activation(std[:], mv[:, :, 1], AF.Sqrt, bias=self.epsA[:, 0:1])
        nc.vector.reciprocal(rstd[:], std[:])
        # --- xc = x - mean (bf16) ---
        xc = self.p1.tile([P, TN, D], BF16, tag="QT")
        for tn in range(TN):
            nc.vector.tensor_scalar(
                out=xc[:, tn, :], in0=self.x[:, tn, :],
                scalar1=negmean[:, tn:tn + 1], scalar2=None, op0=ALU.add)
        # --- transpose via PE with diag(rstd) as rhs; fold w,b on evac ---
        hT = self.p1.tile([P, DN, cfg.T], BF16, tag=tag_out)
        for tn in range(TN):
            diag = self.p2.tile([P, P], BF16, tag="diag")
            nc.vector.tensor_scalar_mul(diag[:], self.identF[:], rstd[:, tn:tn + 1])
            for dn in range(DN):
                ps = self.ps_tr.tile([P, P], F32, tag="tr")
                nc.tensor.matmul(ps[:], lhsT=xc[:, tn, dn * P:(dn + 1) * P],
                                 rhs=diag[:], start=True, stop=True)
                nc.vector.tensor_scalar(
                    out=hT[:, dn, tn * P:(tn + 1) * P], in0=ps[:],
                    scalar1=w_col[:, dn:dn + 1], scalar2=b_col[:, dn:dn + 1],
                    op0=ALU.mult, op1=ALU.add)
        if tapname:
            self.tap(tapname, hT[:])
        return hT

    def load_w(self, dram_ap, shape, tag="w"):
        t = self.p3.tile(shape, BF16, tag=tag)
        self.nc.sync.dma_start(out=t[:], in_=dram_ap)
        return t

    def emit_qkT(self, hT, w_sb, bias_col, scale, out_tag, tapname=None):
        """QT/KT [P, DN, T] = (h @ W + b)^T * scale, bias/scale per-partition."""
        nc, cfg = self.nc, self.cfg
        out = self.p1.tile([P, cfg.DN, cfg.T], BF16, tag=out_tag)
        for cn in range(cfg.DN):
            for (js, jw) in _chunks(cfg.T, cfg.TQC):
                ps = self.ps_mm.tile([P, cfg.TQC], F32, tag="mm")
                for kn in range(cfg.DN):
                    nc.tensor.matmul(
                        ps[:, :jw], lhsT=w_sb[:, kn, cn * P:(cn + 1) * P],
                        rhs=hT[:, kn, js:js + jw],
                        start=(kn == 0), stop=(kn == cfg.DN - 1))
                if scale is None:
                    nc.vector.tensor_scalar(
                        out=out[:, cn, js:js + jw], in0=ps[:, :jw],
                        scalar1=bias_col[:, cn:cn + 1], scalar2=None, op0=ALU.add)
                else:
                    nc.vector.tensor_scalar(
                        out=out[:, cn, js:js + jw], in0=ps[:, :jw],
                        scalar1=bias_col[:, cn:cn + 1], scalar2=scale,
                        op0=ALU.add, op1=ALU.mult)
        if tapname:
            self.tap(tapname, out[:])
        return out

    def emit_v(self, hT, w_sb, bvrow_sb, tapname=None):
        """Vaug [P, TN, H, DH+1] bf16: v tokens-on-partitions + ones column."""
        nc, cfg = self.nc, self.cfg
        DH = cfg.DH
        vaug = self.p1.tile([P, cfg.TN, cfg.H, DH + 1], BF16, tag="vaug")
        nc.vector.memset(vaug[:, :, :, DH:DH + 1], 1.0)
        hpc = cfg.TQC // DH  # heads per c-chunk
        for tn in range(cfg.TN):
            for (cs, cw) in _chunks(cfg.D, cfg.TQC):
                ps = self.ps_mm.tile([P, cfg.TQC], F32, tag="mm")
                for kn in range(cfg.DN):
                    nc.tensor.matmul(
                        ps[:, :cw], lhsT=hT[:, kn, tn * P:(tn + 1) * P],
                        rhs=w_sb[:, kn, cs:cs + cw],
                        start=(kn == 0), stop=False)
                nc.tensor.matmul(
                    ps[:, :cw], lhsT=self.ones_row[0:1, :],
                    rhs=bvrow_sb[0:1, cs:cs + cw], start=False, stop=True)
                h0 = cs // DH
                nc.vector.tensor_copy(
                    out=vaug[:, tn, h0:h0 + hpc, 0:DH],
                    in_=ps[:, :cw].rearrange("p (h e) -> p h e", e=DH))
        if tapname:
            self.tap(tapname, vaug[:])
        return vaug

    def emit_attention(self, QT, KT, vaug, mask_sb):
        """Returns ctxTn [P, DN, T] bf16 (normalized ctx^T)."""
        nc, cfg = self.nc, self.cfg
        TN, DH, H = cfg.TN, cfg.DH, cfg.H
        ctxTu = self.p1.tile([64, H, cfg.T], BF16, tag="big32")
        den = self.p1.tile([65, cfg.T], F32, tag="den")
        dtok = self.ps_den.tile([P, P], F32, tag="dentok")
        n_dcol = 0
        for h in range(H):
            p0 = (h % cfg.HPB) * DH
            dn_h = h // cfg.HPB
            pctx = [self.ps_ctx.tile([P, cfg.TQC], F32, tag="ctx",
                                     name=f"pctx{j}")
                    for j in range(cfg.NJ)]
            for i in range(TN):
                # scores^T for tk-tile i: [P(tk), T(tq)]
                prod = self.p2.tile([P, cfg.T], F32, tag="prod")
                for j, (js, jw) in enumerate(_chunks(cfg.T, cfg.TQC)):
                    ps = self.ps_mm.tile([P, cfg.TQC], F32, tag="mm")
                    nc.tensor.matmul(
                        ps[:, :jw],
                        lhsT=KT[p0:p0 + DH, dn_h, i * P:(i + 1) * P],
                        rhs=QT[p0:p0 + DH, dn_h, js:js + jw],
                        start=True, stop=True)
                    nc.vector.tensor_tensor(
                        out=prod[:, js:js + jw], in0=ps[:, :jw],
                        in1=mask_sb[:, i, js:js + jw], op=ALU.mult)
                expT = self.p2.tile([P, cfg.T], BF16, tag="expT")
                nc.scalar.activation(expT[:], prod[:], AF.Exp)
                for j, (js, jw) in enumerate(_chunks(cfg.T, cfg.TQC)):
                    nc.tensor.matmul(
                        pctx[j][:DH + 1, :jw], lhsT=vaug[:, i, h, :],
                        rhs=expT[:, js:js + jw],
                        start=(i == 0), stop=(i == TN - 1))
            for j, (js, jw) in enumerate(_chunks(cfg.T, cfg.TQC)):
                nc.vector.tensor_copy(out=ctxTu[:, h, js:js + jw],
                                      in_=pctx[j][0:DH, :jw])
                nc.vector.tensor_copy(out=den[64:65, js:js + jw],
                                      in_=pctx[j][DH:DH + 1, :jw])
            for ts in range(TN):
                nc.tensor.matmul(
                    dtok[:, n_dcol:n_dcol + 1],
                    lhsT=den[64:65, ts * P:(ts + 1) * P],
                    rhs=self.onesF[64:65, 0:1], start=True, stop=True)
                n_dcol += 1
        recip = self.p1.tile([P, H * TN], F32, tag="recip")
        nc.vector.reciprocal(recip[:], dtok[:, :H * TN])
        self.tap("recip0", recip[:])

        # T1: ctxTu -> token layout, normalizing per token (partition scalar)
        ctxn = self.p1.tile([P, TN, cfg.D], BF16, tag="QT")
        for h in range(H):
            for ts in range(TN):
                ps = self.ps_tr.tile([P, P], F32, tag="tr")
                nc.tensor.matmul(ps[:, :DH],
                                 lhsT=ctxTu[:, h, ts * P:(ts + 1) * P],
                                 rhs=self.identB[0:64, 0:DH],
                                 start=True, stop=True)
                nc.vector.tensor_scalar(
                    out=ctxn[:, ts, h * DH:(h + 1) * DH], in0=ps[:, :DH],
                    scalar1=recip[:, h * TN + ts:h * TN + ts + 1],
                    scalar2=None, op0=ALU.mult)
        self.tap("ctxn0", ctxn[:])
        # T2: token layout -> ctxTn [P, DN, T]
        ctxTn = self.p1.tile([P, cfg.DN, cfg.T], BF16, tag="KT")
        for ts in range(TN):
            for dn in range(cfg.DN):
                ps = self.ps_tr.tile([P, P], F32, tag="tr")
                nc.tensor.matmul(ps[:], lhsT=ctxn[:, ts, dn * P:(dn + 1) * P],
                                 rhs=self.identB[:], start=True, stop=True)
                nc.vector.tensor_copy(out=ctxTn[:, dn, ts * P:(ts + 1) * P],
                                      in_=ps[:])
        return ctxTn

    def emit_proj_residual(self, srcT, w_sb, brow_sb, kn_list=None, w_kn_of=None):
        """x += srcT^T @ W + b_row.  srcT [P, DN, T], W [P, DN, D]-style."""
        nc, cfg = self.nc, self.cfg
        if kn_list is None:
            kn_list = list(range(cfg.DN))
        for tn in range(cfg.TN):
            for (cs, cw) in _chunks(cfg.D, cfg.TQC):
                ps = self.ps_mm.tile([P, cfg.TQC], F32, tag="mm")
                for ki, kn in enumerate(kn_list):
                    wt, wkn = (w_sb, kn) if w_kn_of is None else w_kn_of(kn)
                    nc.tensor.matmul(
                        ps[:, :cw], lhsT=srcT[:, kn, tn * P:(tn + 1) * P],
                        rhs=wt[:, wkn, cs:cs + cw],
                        start=(ki == 0), stop=False)
                nc.tensor.matmul(
                    ps[:, :cw], lhsT=self.ones_row[0:1, :],
                    rhs=brow_sb[0:1, cs:cs + cw], start=False, stop=True)
                nc.vector.tensor_add(out=self.x[:, tn, cs:cs + cw],
                                     in0=self.x[:, tn, cs:cs + cw], in1=ps[:, :cw])

    def emit_mlp(self, l):
        nc, cfg = self.nc, self.cfg
        ln2w = self.p2.tile([P, cfg.DN], F32, tag="lncol")
        ln2b = self.p2.tile([P, cfg.DN], F32, tag="lncol2")
        nc.sync.dma_start(out=ln2w[:], in_=self.d_ln2w[l])
        nc.sync.dma_start(out=ln2b[:], in_=self.d_ln2b[l])
        hT = self.emit_ln_to_hT(ln2w, ln2b, tapname=("h2T0" if l == 0 else None))
        b1 = self.p2.tile([P, cfg.FFN], F32, tag="b1col")
        nc.sync.dma_start(out=b1[:], in_=self.d_b1[l])
        b2row = self.p1.tile([1, cfg.D], BF16, tag="brow")
        nc.sync.dma_start(out=b2row[:], in_=self.d_b2row[l])

        FO_H = min(cfg.FFN, 16)          # ff 128-tiles per half
        n_half = (cfg.FFN + FO_H - 1) // FO_H
        W1CW = min(1024, FO_H * P)       # w1 column chunk
        W2KN = min(8, FO_H)              # w2 kn-tiles per load chunk
        for half in range(n_half):
            fo0 = half * FO_H
            gT = self.p1.tile([P, FO_H, cfg.T], BF16, tag="big32")
            for (ws, ww) in _chunks(FO_H * P, W1CW):
                w1t = self.load_w(
                    self.d_w1[l][:, :, fo0 * P + ws: fo0 * P + ws + ww],
                    [P, cfg.DN, ww], tag="w")
                for fi in range(ww // P):
                    fo = (ws + fi * P) // P
                    for (js, jw) in _chunks(cfg.T, cfg.TQC):
                        ps = self.ps_mm.tile([P, cfg.TQC], F32, tag="mm")
                        for kn in range(cfg.DN):
                            nc.tensor.matmul(
                                ps[:, :jw],
                                lhsT=w1t[:, kn, fi * P:(fi + 1) * P],
                                rhs=hT[:, kn, js:js + jw],
                                start=(kn == 0), stop=(kn == cfg.DN - 1))
                        nc.scalar.activation(
                            gT[:, fo, js:js + jw], ps[:, :jw], AF.Gelu,
                            bias=b1[:, fo0 + fo:fo0 + fo + 1])
            if l == 0 and half == 0:
                self.tap("gT0", gT[:])
            # y += gT^T @ W2[half rows]
            w2ts = []
            for (ks, kw) in _chunks(FO_H, W2KN):
                w2ts.append((ks, self.load_w(
                    self.d_w2[l][:, fo0 + ks: fo0 + ks + kw, :],
                    [P, kw, cfg.D], tag="w")))

            def w_kn_of(kn):
                for ks, wt in w2ts:
                    if ks <= kn < ks + wt.shape[1]:
                        return wt, kn - ks
                raise AssertionError

            # bias row only on the last half (added once)
            if half == n_half - 1:
                brow = b2row
            else:
                brow = self.p1.tile([1, cfg.D], BF16, tag="zrow")
                nc.vector.memset(brow[:], 0.0)
            self.emit_proj_residual(gT, None, brow,
                                    kn_list=list(range(FO_H)), w_kn_of=w_kn_of)

    def emit_attn_pass(self, l, m):
        nc, cfg = self.nc, self.cfg
        first = (l == 0 and m == 0)
        ln1w = self.p2.tile([P, cfg.DN], F32, tag="lncol")
        ln1b = self.p2.tile([P, cfg.DN], F32, tag="lncol2")
        nc.sync.dma_start(out=ln1w[:], in_=self.d_ln1w[l])
        nc.sync.dma_start(out=ln1b[:], in_=self.d_ln1b[l])
        bq = self.p2.tile([P, cfg.DN], F32, tag="bqcol")
        bk = self.p2.tile([P, cfg.DN], F32, tag="bkcol")
        nc.sync.dma_start(out=bq[:], in_=self.d_bq[l])
        nc.sync.dma_start(out=bk[:], in_=self.d_bk[l])
        bvrow = self.p1.tile([1, cfg.D], BF16, tag="brow")
        nc.sync.dma_start(out=bvrow[:], in_=self.d_bvrow[l])
        borow = self.p1.tile([1, cfg.D], BF16, tag="brow2")
        nc.sync.dma_start(out=borow[:], in_=self.d_borow[l])
        mask_sb = self.p1.tile([P, cfg.TN, cfg.T], BF16, tag="mask")
        nc.sync.dma_start(out=mask_sb[:], in_=self.d_maskT[m])

        hT = self.emit_ln_to_hT(ln1w, ln1b, tapname=("hT0" if first else None))
        scale = 1.0 / math.sqrt(cfg.DH)
        wq = self.load_w(self.d_wq[l][:], [P, cfg.DN, cfg.D], tag="w")
        QT = self.emit_qkT(hT, wq, bq, scale, "QT", tapname=("QT0" if first else None))
        wk = self.load_w(self.d_wk[l][:], [P, cfg.DN, cfg.D], tag="w")
        KT = self.emit_qkT(hT, wk, bk, None, "KT", tapname=("KT0" if first else None))
        wv = self.load_w(self.d_wv[l][:], [P, cfg.DN, cfg.D], tag="w")
        vaug = self.emit_v(hT, wv, bvrow, tapname=("V0" if first else None))
        ctxTn = self.emit_attention(QT, KT, vaug, mask_sb)
        wo = self.load_w(self.d_wo[l][:], [P, cfg.DN, cfg.D], tag="w")
        self.emit_proj_residual(ctxTn, wo, borow)
        if first:
            self.tap("xp0", self.x[:])

    def emit_head(self):
        nc, cfg = self.nc, self.cfg
        lnfw = self.p2.tile([P, cfg.DN], F32, tag="lncol")
        lnfb = self.p2.tile([P, cfg.DN], F32, tag="lncol2")
        nc.sync.dma_start(out=lnfw[:], in_=self.d_lnfw[:])
        nc.sync.dma_start(out=lnfb[:], in_=self.d_lnfb[:])
        xfT = self.emit_ln_to_hT(lnfw, lnfb, tapname="xfT")
        for (vs, vw) in _chunks(cfg.VS, 1024):
            hw = self.load_w(self.d_head[:, :, vs:vs + vw], [P, cfg.DN, vw], tag="w")
            for tn in range(cfg.TN):
                for (js, jw) in _chunks(vw, 512):
                    ps = self.ps_mm.tile([P, 512], F32, tag="mm")
                    for kn in range(cfg.DN):
                        nc.tensor.matmul(
                            ps[:, :jw], lhsT=xfT[:, kn, tn * P:(tn + 1) * P],
                            rhs=hw[:, kn, js:js + jw],
                            start=(kn == 0), stop=(kn == cfg.DN - 1))
                    lg = self.p2.tile([P, 512], F32, tag="prod")
                    nc.vector.tensor_copy(out=lg[:, :jw], in_=ps[:, :jw])
                    nc.sync.dma_start(
                        out=self.d_out[tn * P:(tn + 1) * P, vs + js:vs + js + jw],
                        in_=lg[:, :jw])

    # ---------------- top level ----------------
    def build(self):
        self.declare_params()
        with ExitStack() as ctx:
            self.tc = ctx.enter_context(tile.TileContext(self.nc))
            self.open_pools(ctx)
            self.emit_constants()
            self.emit_embedding()
            for l in range(self.cfg.L):
                for m in range(self.cfg.NM):
                    self.emit_attn_pass(l, m)
                self.emit_mlp(l)
                if l == 0:
                    self.tap("xl0", self.x[:])
            self.tap("xf", self.x[:])
            self.emit_head()
        self.nc.finalize()  # bacc: register allocation + codegen passes
        return self.nc


# ---------------- host-side packing ----------------
def _bf(a):
    return np.asarray(a, dtype=np.float32).astype(ml_dtypes.bfloat16)


def _r3(w, pdim=P):
    """[K, N] -> [P, K//P, N] with K = kn*P + kp."""
    K, N = w.shape
    return np.ascontiguousarray(w.reshape(K // pdim, pdim, N).transpose(1, 0, 2))


def _rcol(v):
    """[K] -> [P, K//P] (k = kn*P + kp)."""
    return np.ascontiguousarray(v.reshape(-1, P).T)


def pack_shared(cfg: Cfg, inp):
    """Everything identical across cores."""
    sh = {}
    sh["tok_emb"] = np.ascontiguousarray(np.asarray(inp["tok_emb"], np.float32))
    pos = np.asarray(inp["pos_emb"], np.float32)[0]  # [T, D]
    sh["pos_r"] = _r3(pos)  # [P, TN, D]
    m = np.asarray(inp["masks"], np.float32)
    mT = m.transpose(0, 2, 1)  # [NM, tk, tq]
    sh["masksT_r"] = np.ascontiguousarray(
        _bf(mT).reshape(cfg.NM, cfg.TN, P, cfg.T).transpose(0, 2, 1, 3))
    for name, key in (("wq_r", "Wq"), ("wk_r", "Wk"), ("wv_r", "Wv"),
                      ("wo_r", "Wo"), ("w1_r", "W1"), ("w2_r", "W2")):
        w = _bf(inp[key])
        sh[name] = np.ascontiguousarray(
            w.reshape(cfg.L, w.shape[1] // P, P, w.shape[2]).transpose(0, 2, 1, 3))
    for name, key in (("ln1w_r", "ln1_w"), ("ln1b_r", "ln1_b"),
                      ("ln2w_r", "ln2_w"), ("ln2b_r", "ln2_b"),
                      ("bq_r", "bq"), ("bk_r", "bk")):
        v = np.asarray(inp[key], np.float32)
        sh[name] = np.ascontiguousarray(
            v.reshape(cfg.L, -1, P).transpose(0, 2, 1))
    sh["b1_r"] = np.ascontiguousarray(
        np.asarray(inp["b1"], np.float32).reshape(cfg.L, -1, P).transpose(0, 2, 1))
    sh["bv_row"] = np.ascontiguousarray(_bf(inp["bv"])[:, None, :])
    sh["bo_row"] = np.ascontiguousarray(_bf(inp["bo"])[:, None, :])
    sh["b2_row"] = np.ascontiguousarray(_bf(inp["b2"])[:, None, :])
    sh["lnfw_r"] = _rcol(np.asarray(inp["lnf_w"], np.float32))
    sh["lnfb_r"] = _rcol(np.asarray(inp["lnf_b"], np.float32))
    return sh


def pack_core(cfg: Cfg, inp, sh, b, half, head_halves):
    m = dict(sh)
    idx = np.asarray(inp["idx"]).astype(np.int32)[b]  # [T]
    m["idx"] = np.ascontiguousarray(idx[:, None])
    m["head_r"] = head_halves[half]
    return m


def prepare(inputs, cfg=None):
    """Build the SPMD program and the 8 per-core input maps."""
    cfg = cfg or Cfg()
    nc = GPTBuilder(cfg).build()
    sh = pack_shared(cfg, inputs)
    hw = np.asarray(inputs["head_w"], np.float32)
    hpad = np.zeros((cfg.D, 2 * cfg.VS), np.float32)
    hpad[:, :cfg.V] = hw
    head_halves = [
        np.ascontiguousarray(_r3(_bf(hpad[:, i * cfg.VS:(i + 1) * cfg.VS])))
        for i in range(2)
    ]
    in_maps = [pack_core(cfg, inputs, sh, c // 2, c % 2, head_halves)
               for c in range(N_CORES)]
    return nc, in_maps


def assemble(cfg, results):
    logits = np.empty((B_FULL, cfg.T, cfg.V), np.float32)
    for b in range(B_FULL):
        lo = results[2 * b]["out"]
        hi = results[2 * b + 1]["out"]
        full = np.concatenate([lo, hi], axis=1)
        logits[b] = full[:, :cfg.V]
    return logits


def kernel(**inputs) -> np.ndarray:
    from concourse.bass_utils import run_bass_kernel_spmd

    cfg = Cfg()
    nc, in_maps = prepare(inputs, cfg)
    res = run_bass_kernel_spmd(nc, in_maps, list(range(N_CORES)))
    return assemble(cfg, res.results)


# revision 23
# speedup vs baseline: 2.1041x; 2.1041x over previous
"""Trainium2 Bass kernel for a small GPT (multi-head attention with
multiplicative masks, applied NM times per layer, + MLP, + vocab head).

Sharding over 8 NeuronCores (SPMD, zero collectives):
  core c -> batch element b = c // 2   (each batch element's transformer body
            is computed redundantly on a pair of cores),
            vocab shard     h = c % 2  (the LM head weight is split in two
            along the vocab dim; each core of the pair emits logits for its
            half of the (padded) vocabulary for all T tokens of its batch
            element).
The per-core program is identical; only input data differs (SPMD).

All matmuls run in bf16 with fp32 PSUM accumulation; the residual stream,
layernorm statistics and softmax denominators stay in fp32.

Internal layouts (SBUF, partition dim first, P=128):
  x      [P, TN, D]  fp32   token-partition residual stream, t = tn*P + tp
  hT     [P, DN, T]  bf16   LN output, transposed: hT[dp, dn, t] = h[t, dn*P+dp]
  QT/KT  [P, DN, T]  bf16   q/k transposed; head h lives on partitions
                            (h%2)*64..(h%2)*64+64 at dn = h//2
  Vaug   [P, TN, H, DH+1] bf16  v in token layout + ones column (col DH) so the
                            ctx matmul also produces softmax denominators
  expT   [P, T]      bf16   exp(mask * scores^T) for one tk-tile (streamed)
  ctxTu  [64, H, T]  bf16   unnormalized ctx^T per head
  ctxn   [P, TN, D]  bf16   normalized ctx, token layout (after transpose T1)
  ctxTn  [P, DN, T]  bf16   normalized ctx^T (after transpose T2) for out-proj
"""

import math
from contextlib import ExitStack
from dataclasses import dataclass

import numpy as np
import ml_dtypes

import concourse.bass as bass
import concourse.mybir as mybir
import concourse.tile as tile
from concourse import bacc
from concourse.masks import make_identity

F32 = mybir.dt.float32
BF16 = mybir.dt.bfloat16
I32 = mybir.dt.int32
AF = mybir.ActivationFunctionType
ALU = mybir.AluOpType
P = 128

# ---------------- model dims (from the reference problem) ----------------
B_FULL, T_FULL, D_FULL, H_FULL, L_FULL = 4, 1024, 1024, 16, 6
V_FULL, NM_FULL, DFF_FULL = 50257, 2, 4 * 1024
VS_FULL = 25600  # per-core padded vocab shard (2*25600 = 51200 >= 50257)
N_CORES = 8


@dataclass(frozen=True)
class Cfg:
    T: int = T_FULL
    D: int = D_FULL
    H: int = H_FULL
    DH: int = 64
    L: int = L_FULL
    NM: int = NM_FULL
    DFF: int = DFF_FULL
    V: int = V_FULL
    VS: int = VS_FULL
    eps: float = 1e-5
    debug_taps: tuple = ()
    nz: tuple = ("bv", "bo", "b2")  # which rank-1 biases to emit

    @property
    def TN(self):
        return self.T // P

    @property
    def DN(self):
        return self.D // P

    @property
    def FFN(self):
        return self.DFF // P

    tqc0: int = 512

    @property
    def TQC(self):  # tq/free-dim chunk size for matmul N (psum bank = 512 f32)
        return min(self.tqc0, self.T)

    @property
    def NJ(self):
        return self.T // self.TQC

    @property
    def HPB(self):  # heads per 128-partition block
        return P // self.DH


def _chunks(total, w):
    return [(s, min(w, total - s)) for s in range(0, total, w)]


class GPTBuilder:
    def __init__(self, cfg: Cfg):
        self.cfg = cfg
        self.nc = bacc.Bacc("TRN2", target_bir_lowering=False, debug=False)
        self.taps = {}

    # ---------------- dram params ----------------
    def declare_params(self):
        nc, cfg = self.nc, self.cfg
        dt = nc.dram_tensor
        self.d_idx = dt("idx", [cfg.T, 1], I32, kind="ExternalInput")
        self.d_tok = dt("tok_emb", [cfg.V, cfg.D], F32, kind="ExternalInput")
        self.d_pos = dt("pos_r", [P, cfg.TN, cfg.D], F32, kind="ExternalInput")
        self.d_maskT = dt("masksT_r", [cfg.NM, P, cfg.TN, cfg.T], BF16,
                          kind="ExternalInput")
        self.d_wq = dt("wq_r", [cfg.L, P, cfg.DN, cfg.D], BF16, kind="ExternalInput")
        self.d_wk = dt("wk_r", [cfg.L, P, cfg.DN, cfg.D], BF16, kind="ExternalInput")
        self.d_wv = dt("wv_r", [cfg.L, P, cfg.DN, cfg.D], BF16, kind="ExternalInput")
        self.d_wo = dt("wo_r", [cfg.L, P, cfg.DN, cfg.D], BF16, kind="ExternalInput")
        self.d_w1 = dt("w1_r", [cfg.L, P, cfg.DN, cfg.DFF], BF16, kind="ExternalInput")
        self.d_w2 = dt("w2_r", [cfg.L, P, cfg.FFN, cfg.D], BF16, kind="ExternalInput")
        self.d_ln1w = dt("ln1w_r", [cfg.L, P, cfg.DN], F32, kind="ExternalInput")
        self.d_ln1b = dt("ln1b_r", [cfg.L, P, cfg.DN], F32, kind="ExternalInput")
        self.d_ln2w = dt("ln2w_r", [cfg.L, P, cfg.DN], F32, kind="ExternalInput")
        self.d_ln2b = dt("ln2b_r", [cfg.L, P, cfg.DN], F32, kind="ExternalInput")
        self.d_bq = dt("bq_r", [cfg.L, P, cfg.DN], F32, kind="ExternalInput")
        self.d_bk = dt("bk_r", [cfg.L, P, cfg.DN], F32, kind="ExternalInput")
        self.d_b1 = dt("b1_r", [cfg.L, P, cfg.FFN], F32, kind="ExternalInput")
        self.d_bvrow = dt("bv_row", [cfg.L, 1, cfg.D], BF16, kind="ExternalInput")
        self.d_borow = dt("bo_row", [cfg.L, 1, cfg.D], BF16, kind="ExternalInput")
        self.d_b2row = dt("b2_row", [cfg.L, 1, cfg.D], BF16, kind="ExternalInput")
        self.d_lnfw = dt("lnfw_r", [P, cfg.DN], F32, kind="ExternalInput")
        self.d_lnfb = dt("lnfb_r", [P, cfg.DN], F32, kind="ExternalInput")
        self.d_head = dt("head_r", [P, cfg.DN, cfg.VS], BF16, kind="ExternalInput")
        self.d_out = dt("out", [cfg.T, cfg.VS], F32, kind="ExternalOutput")

    def tap(self, name, ap, dtype=None):
        """Optionally expose an SBUF tile as an extra output (debug)."""
        if name not in self.cfg.debug_taps or name in self.taps:
            return
        nc = self.nc
        dt = dtype or ap.dtype
        d = nc.dram_tensor(f"tap_{name}", list(ap.shape), dt, kind="ExternalOutput")
        nc.sync.dma_start(out=d[:], in_=ap)
        self.taps[name] = d

    # ---------------- pools ----------------
    def open_pools(self, ctx: ExitStack):
        tc = self.tc
        self.p1 = ctx.enter_context(tc.tile_pool(name="p1", bufs=1))
        self.p2 = ctx.enter_context(tc.tile_pool(name="p2", bufs=2))
        self.p3 = ctx.enter_context(tc.tile_pool(name="p3", bufs=2))
        self.p4 = ctx.enter_context(tc.tile_pool(name="p4", bufs=3))
        self.ps_mm = ctx.enter_context(tc.tile_pool(name="ps_mm", bufs=2, space="PSUM"))
        self.ps_ctx = ctx.enter_context(tc.tile_pool(name="ps_ctx", bufs=2, space="PSUM"))
        self.ps_tr = ctx.enter_context(tc.tile_pool(name="ps_tr", bufs=2, space="PSUM"))

    # ---------------- building blocks ----------------
    def emit_constants(self):
        nc, cfg = self.nc, self.cfg
        self.identF = self.p1.tile([P, P], F32, tag="identF")
        make_identity(nc, self.identF[:])
        self.identB = self.p1.tile([P, P], BF16, tag="identB")
        nc.vector.tensor_copy(out=self.identB[:], in_=self.identF[:])
        self.onesB = self.p1.tile([P, 1], BF16, tag="onesB")
        nc.vector.memset(self.onesB[:], 1.0)
        self.ones_row = self.p1.tile([1, P], BF16, tag="ones_row")
        nc.vector.memset(self.ones_row[:], 1.0)
        self.epsA = self.p1.tile([P, 1], F32, tag="epsA")
        nc.vector.memset(self.epsA[:], cfg.eps)
        self.onesF = self.p1.tile([P, 1], F32, tag="onesF")
        nc.vector.memset(self.onesF[:], 1.0)

    def emit_embedding(self):
        nc, cfg = self.nc, self.cfg
        self.x = self.p1.tile([P, cfg.TN, cfg.D], F32, tag="x")
        pos = self.p1.tile([P, cfg.TN, cfg.D], F32, tag="big32")
        nc.sync.dma_start(out=pos[:], in_=self.d_pos[:])
        for tn in range(cfg.TN):
            it = self.p2.tile([P, 1], I32, tag="idx")
            nc.sync.dma_start(out=it[:], in_=self.d_idx[tn * P:(tn + 1) * P, :])
            nc.gpsimd.indirect_dma_start(
                out=self.x[:, tn, :], out_offset=None,
                in_=self.d_tok[:],
                in_offset=bass.IndirectOffsetOnAxis(ap=it[:, :1], axis=0),
            )
        nc.vector.tensor_add(out=self.x[:], in0=self.x[:], in1=pos[:])
        self.tap("x0", self.x[:])

    def emit_ln_to_hT(self, w_col, b_col, tag_out="hT", tapname=None):
        """LayerNorm(x) -> transposed hT [P, DN, T] bf16.

        w_col/b_col: [P, DN] fp32 SBUF tiles (per-d scale/bias, d on partitions
        in the transposed layout, so they fold into the PSUM evacuation).
        """
        nc, cfg = self.nc, self.cfg
        TN, DN, D = cfg.TN, cfg.DN, cfg.D
        # --- stats: bn_stats/bn_aggr per token tile -> mean, var ---
        ngrp = max(1, D // 512)
        gsz = D // ngrp
        mv = self.p1.tile([P, TN, 2], F32, tag="mv")
        for tn in range(TN):
            bnst = self.p2.tile([P, ngrp, 6], F32, tag="bnst")
            for g in range(ngrp):
                nc.vector.bn_stats(out=bnst[:, g, :],
                                   in_=self.x[:, tn, g * gsz:(g + 1) * gsz])
            nc.vector.bn_aggr(out=mv[:, tn, :], in_=bnst[:])
        negmean = self.p1.tile([P, TN], F32, tag="negmean")
        rstd = self.p1.tile([P, TN], F32, tag="rstd")
        std = self.p1.tile([P, TN], F32, tag="std")
        nc.vector.tensor_scalar_mul(negmean[:], mv[:, :, 0], -1.0)
        # std = sqrt(var + eps); rstd = 1/std  (accurate reciprocal on DVE)
        nc.scalar.activation(std[:], mv[:, :, 1], AF.Sqrt, bias=self.epsA[:, 0:1])
        nc.vector.reciprocal(rstd[:], std[:])
        # --- xc = x - mean (bf16) ---
        xc = self.p1.tile([P, TN, D], BF16, tag="QT")
        for tn in range(TN):
            nc.vector.tensor_scalar(
                out=xc[:, tn, :], in0=self.x[:, tn, :],
                scalar1=negmean[:, tn:tn + 1], scalar2=None, op0=ALU.add)
        # --- transpose via PE with diag(rstd) as rhs; fold w,b on evac ---
        hT = self.p1.tile([P, DN, cfg.T], BF16, tag=tag_out)
        for tn in range(TN):
            diag = self.p2.tile([P, P], BF16, tag="diag")
            nc.vector.tensor_scalar_mul(diag[:], self.identF[:], rstd[:, tn:tn + 1])
            for dn in range(DN):
                ps = self.ps_tr.tile([P, P], F32, tag="tr")
                nc.tensor.matmul(ps[:], lhsT=xc[:, tn, dn * P:(dn + 1) * P],
                                 rhs=diag[:], start=True, stop=True)
                nc.scalar.activation(
                    hT[:, dn, tn * P:(tn + 1) * P], ps[:], AF.Identity,
                    bias=b_col[:, dn:dn + 1], scale=w_col[:, dn:dn + 1])
        if tapname:
            self.tap(tapname, hT[:])
        return hT

    def load_w(self, dram_ap, shape, tag="w"):
        t = self.p3.tile(shape, BF16, tag=tag)
        self.nc.sync.dma_start(out=t[:], in_=dram_ap)
        return t

    def emit_qkT(self, hT, w_sb, bias_col, scale, out_tag, tapname=None):
        """QT/KT [P, DN, T] = (h @ W)^T * scale + bias_col.

        NOTE: the caller must pre-scale bias_col by `scale` (ACT computes
        func(in*scale + bias)). Evacuation runs on ACT (Identity).
        """
        nc, cfg = self.nc, self.cfg
        out = self.p1.tile([P, cfg.DN, cfg.T], BF16, tag=out_tag)
        jl = _chunks(cfg.T, cfg.TQC)
        jw = cfg.TQC
        for cn in range(cfg.DN):
            ps = self.ps_mm.tile([P, len(jl), 512], F32, tag="mm",
                                 name="ps_qk")
            for kn in range(cfg.DN):
                for j, (js, _) in enumerate(jl):
                    nc.tensor.matmul(
                        ps[:, j, :jw], lhsT=w_sb[:, kn, cn * P:(cn + 1) * P],
                        rhs=hT[:, kn, js:js + jw],
                        start=(kn == 0), stop=(kn == cfg.DN - 1),
                        skip_group_check=True)
            nc.scalar.activation(
                out[:, cn, :].rearrange("p (j w) -> p j w", w=jw),
                ps[:, :, :jw], AF.Identity,
                bias=bias_col[:, cn:cn + 1], scale=(1.0 if scale is None else scale))
        if tapname:
            self.tap(tapname, out[:])
        return out

    def emit_v(self, hT, w_sb, bvrow_sb, tapname=None):
        """Vaug [P, TN, H, DH+1] bf16: v tokens-on-partitions + ones column."""
        nc, cfg = self.nc, self.cfg
        DH = cfg.DH
        use_bias = "bv" in cfg.nz
        vaug = self.p1.tile([P, cfg.TN, cfg.H, DH + 1], BF16, tag="vaug")
        nc.vector.memset(vaug[:, :, :, DH:DH + 1], 1.0)
        cl = _chunks(cfg.D, cfg.TQC)
        cw = cfg.TQC
        for tn in range(cfg.TN):
            ps = self.ps_mm.tile([P, len(cl), 512], F32, tag="mm",
                                 name="ps_v")
            for kn in range(cfg.DN):
                for j, (cs, _) in enumerate(cl):
                    nc.tensor.matmul(
                        ps[:, j, :cw], lhsT=hT[:, kn, tn * P:(tn + 1) * P],
                        rhs=w_sb[:, kn, cs:cs + cw],
                        start=(kn == 0),
                        stop=(kn == cfg.DN - 1 and not use_bias),
                        skip_group_check=True)
            if use_bias:
                for j, (cs, _) in enumerate(cl):
                    nc.tensor.matmul(
                        ps[:, j, :cw], lhsT=self.ones_row[0:1, :],
                        rhs=bvrow_sb[0:1, cs:cs + cw], start=False, stop=True,
                        skip_group_check=True)
            nc.vector.tensor_copy(
                out=vaug[:, tn, :, 0:DH].rearrange("p (j h) e -> p j h e",
                                                   j=len(cl)),
                in_=ps[:, :, :cw].rearrange("p j (h e) -> p j h e", e=DH))
        if tapname:
            self.tap(tapname, vaug[:])
        return vaug

    def emit_attention(self, QT, KT, vaug, mask_sb):
        """Returns ctxTn [P, DN, T] bf16 (normalized ctx^T).

        ctxTuD row 64 carries the softmax denominators (from Vaug's ones
        column); the T1 transpose brings them into token layout, where a
        per-token reciprocal normalizes ctx during the ACT evacuation.
        """
        nc, cfg = self.nc, self.cfg
        TN, DH, H = cfg.TN, cfg.DH, cfg.H
        jl = _chunks(cfg.T, cfg.TQC)
        ctxTuD = self.p1.tile([DH + 1, H, cfg.T], BF16, tag="big32")
        recip = self.p1.tile([P, H * TN], F32, tag="recip")
        ctxn = self.p1.tile([P, TN, cfg.D], BF16, tag="QT")
        jw = cfg.TQC
        for h in range(H):
            p0 = (h % cfg.HPB) * DH
            dn_h = h // cfg.HPB
            pctx = [self.ps_ctx.tile([P, cfg.TQC], F32, tag="ctx",
                                     name=f"pctx{j}")
                    for j in range(cfg.NJ)]
            for i in range(TN):
                # scores^T for tk-tile i: [P(tk), T(tq)], bank-aligned chunks
                ps = self.ps_mm.tile([P, len(jl), 512], F32, tag="mm",
                                      name="ps_sc")
                for j, (js, _) in enumerate(jl):
                    nc.tensor.matmul(
                        ps[:, j, :jw],
                        lhsT=KT[p0:p0 + DH, dn_h, i * P:(i + 1) * P],
                        rhs=QT[p0:p0 + DH, dn_h, js:js + jw],
                        start=True, stop=True)
                prod = self.p4.tile([P, cfg.T], F32, tag="prod")
                nc.vector.tensor_tensor(
                    out=prod[:].rearrange("p (j w) -> p j w", w=jw),
                    in0=ps[:, :, :jw],
                    in1=mask_sb[:, i, :].rearrange("p (j w) -> p j w", w=jw),
                    op=ALU.mult)
                expT = self.p2.tile([P, cfg.T], BF16, tag="expT")
                nc.scalar.activation(expT[:], prod[:], AF.Exp)
                for j, (js, _) in enumerate(jl):
                    nc.tensor.matmul(
                        pctx[j][:DH + 1, :jw], lhsT=vaug[:, i, h, :],
                        rhs=expT[:, js:js + jw],
                        start=(i == 0), stop=(i == TN - 1))
            for j, (js, _) in enumerate(jl):
                nc.vector.tensor_copy(out=ctxTuD[:, h, js:js + jw],
                                      in_=pctx[j][0:DH + 1, :jw])
            # T1 for this head immediately (PE filler during later heads):
            # col DH of the transpose is the softmax denominator.
            for ts in range(TN):
                c = h * TN + ts
                ps = self.ps_tr.tile([P, P], F32, tag="tr")
                nc.tensor.matmul(ps[:, :DH + 1],
                                 lhsT=ctxTuD[:, h, ts * P:(ts + 1) * P],
                                 rhs=self.identB[0:DH + 1, 0:DH + 1],
                                 start=True, stop=True)
                nc.vector.reciprocal(recip[:, c:c + 1], ps[:, DH:DH + 1])
                nc.scalar.activation(
                    ctxn[:, ts, h * DH:(h + 1) * DH], ps[:, :DH], AF.Identity,
                    scale=recip[:, c:c + 1])
        self.tap("recip0", recip[:])
        self.tap("ctxn0", ctxn[:])
        # T2: token layout -> ctxTn [P, DN, T]
        ctxTn = self.p1.tile([P, cfg.DN, cfg.T], BF16, tag="KT")
        for ts in range(TN):
            for dn in range(cfg.DN):
                ps = self.ps_tr.tile([P, P], F32, tag="tr")
                nc.tensor.matmul(ps[:], lhsT=ctxn[:, ts, dn * P:(dn + 1) * P],
                                 rhs=self.identB[:], start=True, stop=True)
                nc.scalar.activation(ctxTn[:, dn, ts * P:(ts + 1) * P],
                                     ps[:], AF.Identity)
        return ctxTn

    def emit_proj_residual(self, srcT, w_sb, brow_sb, kn_list=None,
                           w_kn_of=None, use_bias=True):
        """x += srcT^T @ W (+ b_row).  srcT [P, DN, T], W [P, DN, D]-style."""
        nc, cfg = self.nc, self.cfg
        if kn_list is None:
            kn_list = list(range(cfg.DN))
        cl = _chunks(cfg.D, cfg.TQC)
        cw = cfg.TQC
        for tn in range(cfg.TN):
            ps = self.ps_mm.tile([P, len(cl), 512], F32, tag="mm",
                                 name="ps_pr")
            for ki, kn in enumerate(kn_list):
                wt, wkn = (w_sb, kn) if w_kn_of is None else w_kn_of(kn)
                for j, (cs, _) in enumerate(cl):
                    nc.tensor.matmul(
                        ps[:, j, :cw], lhsT=srcT[:, kn, tn * P:(tn + 1) * P],
                        rhs=wt[:, wkn, cs:cs + cw],
                        start=(ki == 0),
                        stop=(ki == len(kn_list) - 1 and not use_bias),
                        skip_group_check=True)
            if use_bias:
                for j, (cs, _) in enumerate(cl):
                    nc.tensor.matmul(
                        ps[:, j, :cw], lhsT=self.ones_row[0:1, :],
                        rhs=brow_sb[0:1, cs:cs + cw], start=False, stop=True,
                        skip_group_check=True)
            nc.vector.tensor_add(
                out=self.x[:, tn, :].rearrange("p (j w) -> p j w", w=cw),
                in0=self.x[:, tn, :].rearrange("p (j w) -> p j w", w=cw),
                in1=ps[:, :, :cw])

    def emit_mlp(self, l):
        nc, cfg = self.nc, self.cfg
        ln2w = self.p2.tile([P, cfg.DN], F32, tag="lncol")
        ln2b = self.p2.tile([P, cfg.DN], F32, tag="lncol2")
        nc.sync.dma_start(out=ln2w[:], in_=self.d_ln2w[l])
        nc.sync.dma_start(out=ln2b[:], in_=self.d_ln2b[l])
        hT = self.emit_ln_to_hT(ln2w, ln2b, tapname=("h2T0" if l == 0 else None))
        b1 = self.p2.tile([P, cfg.FFN], F32, tag="b1col")
        nc.sync.dma_start(out=b1[:], in_=self.d_b1[l])
        b2row = self.p1.tile([1, cfg.D], BF16, tag="brow")
        nc.sync.dma_start(out=b2row[:], in_=self.d_b2row[l])

        FO_H = min(cfg.FFN, 16)          # ff 128-tiles per half
        n_half = (cfg.FFN + FO_H - 1) // FO_H
        W1CW = min(1024, FO_H * P)       # w1 column chunk
        W2KN = min(8, FO_H)              # w2 kn-tiles per load chunk
        jl = _chunks(cfg.T, cfg.TQC)
        for half in range(n_half):
            fo0 = half * FO_H
            gT = self.p1.tile([P, FO_H, cfg.T], BF16, tag="big32")
            for (ws, ww) in _chunks(FO_H * P, W1CW):
                w1t = self.load_w(
                    self.d_w1[l][:, :, fo0 * P + ws: fo0 * P + ws + ww],
                    [P, cfg.DN, ww], tag="w")
                for fi in range(ww // P):
                    fo = (ws + fi * P) // P
                    jw = cfg.TQC
                    ps = self.ps_mm.tile([P, len(jl), 512], F32, tag="mm",
                                         name="ps_mlp")
                    for kn in range(cfg.DN):
                        for j, (js, _) in enumerate(jl):
                            nc.tensor.matmul(
                                ps[:, j, :jw],
                                lhsT=w1t[:, kn, fi * P:(fi + 1) * P],
                                rhs=hT[:, kn, js:js + jw],
                                start=(kn == 0), stop=(kn == cfg.DN - 1),
                                skip_group_check=True)
                    nc.scalar.activation(
                        gT[:, fo, :].rearrange("p (j w) -> p j w", w=jw),
                        ps[:, :, :jw], AF.Gelu,
                        bias=b1[:, fo0 + fo:fo0 + fo + 1])
            if l == 0 and half == 0:
                self.tap("gT0", gT[:])
            # y += gT^T @ W2[half rows]
            w2ts = []
            for (ks, kw) in _chunks(FO_H, W2KN):
                w2ts.append((ks, self.load_w(
                    self.d_w2[l][:, fo0 + ks: fo0 + ks + kw, :],
                    [P, kw, cfg.D], tag="w")))

            def w_kn_of(kn):
                for ks, wt in w2ts:
                    if ks <= kn < ks + wt.shape[1]:
                        return wt, kn - ks
                raise AssertionError

            use_b2 = ("b2" in cfg.nz) and (half == n_half - 1)
            self.emit_proj_residual(gT, None, b2row,
                                    kn_list=list(range(FO_H)), w_kn_of=w_kn_of,
                                    use_bias=use_b2)

    def emit_attn_pass(self, l, m):
        nc, cfg = self.nc, self.cfg
        first = (l == 0 and m == 0)
        ln1w = self.p2.tile([P, cfg.DN], F32, tag="lncol")
        ln1b = self.p2.tile([P, cfg.DN], F32, tag="lncol2")
        nc.sync.dma_start(out=ln1w[:], in_=self.d_ln1w[l])
        nc.sync.dma_start(out=ln1b[:], in_=self.d_ln1b[l])
        bq = self.p2.tile([P, cfg.DN], F32, tag="bqcol")
        bk = self.p2.tile([P, cfg.DN], F32, tag="bkcol")
        nc.sync.dma_start(out=bq[:], in_=self.d_bq[l])
        nc.sync.dma_start(out=bk[:], in_=self.d_bk[l])
        bvrow = self.p1.tile([1, cfg.D], BF16, tag="brow")
        nc.sync.dma_start(out=bvrow[:], in_=self.d_bvrow[l])
        borow = self.p1.tile([1, cfg.D], BF16, tag="brow2")
        nc.sync.dma_start(out=borow[:], in_=self.d_borow[l])
        mask_sb = self.p1.tile([P, cfg.TN, cfg.T], BF16, tag="mask")
        nc.sync.dma_start(out=mask_sb[:], in_=self.d_maskT[m])

        hT = self.emit_ln_to_hT(ln1w, ln1b, tapname=("hT0" if first else None))
        scale = 1.0 / math.sqrt(cfg.DH)
        wq = self.load_w(self.d_wq[l][:], [P, cfg.DN, cfg.D], tag="w")
        QT = self.emit_qkT(hT, wq, bq, scale, "QT", tapname=("QT0" if first else None))
        wk = self.load_w(self.d_wk[l][:], [P, cfg.DN, cfg.D], tag="w")
        KT = self.emit_qkT(hT, wk, bk, None, "KT", tapname=("KT0" if first else None))
        wv = self.load_w(self.d_wv[l][:], [P, cfg.DN, cfg.D], tag="w")
        vaug = self.emit_v(hT, wv, bvrow, tapname=("V0" if first else None))
        ctxTn = self.emit_attention(QT, KT, vaug, mask_sb)
        wo = self.load_w(self.d_wo[l][:], [P, cfg.DN, cfg.D], tag="w")
        self.emit_proj_residual(ctxTn, wo, borow, use_bias=("bo" in cfg.nz))
        if first:
            self.tap("xp0", self.x[:])

    def emit_head(self):
        nc, cfg = self.nc, self.cfg
        lnfw = self.p2.tile([P, cfg.DN], F32, tag="lncol")
        lnfb = self.p2.tile([P, cfg.DN], F32, tag="lncol2")
        nc.sync.dma_start(out=lnfw[:], in_=self.d_lnfw[:])
        nc.sync.dma_start(out=lnfb[:], in_=self.d_lnfb[:])
        xfT = self.emit_ln_to_hT(lnfw, lnfb, tapname="xfT")
        for (vs, vw) in _chunks(cfg.VS, 1024):
            hw = self.load_w(self.d_head[:, :, vs:vs + vw], [P, cfg.DN, vw], tag="w")
            for tn in range(cfg.TN):
                ps = self.ps_mm.tile([P, 2, 512], F32, tag="mm",
                                     name="ps_hd")
                for kn in range(cfg.DN):
                    for j, (js, jw2) in enumerate(_chunks(vw, 512)):
                        nc.tensor.matmul(
                            ps[:, j, :jw2], lhsT=xfT[:, kn, tn * P:(tn + 1) * P],
                            rhs=hw[:, kn, js:js + jw2],
                            start=(kn == 0), stop=(kn == cfg.DN - 1),
                            skip_group_check=True)
                lg = self.p2.tile([P, 1024], F32, tag="prod")
                nc.scalar.activation(lg[:, :vw], ps[:].rearrange(
                    "p j w -> p (j w)")[:, :vw], AF.Identity)
                nc.sync.dma_start(
                    out=self.d_out[tn * P:(tn + 1) * P, vs:vs + vw],
                    in_=lg[:, :vw])

    # ---------------- top level ----------------
    def build(self):
        self.declare_params()
        with ExitStack() as ctx:
            self.tc = ctx.enter_context(tile.TileContext(self.nc))
            self.open_pools(ctx)
            self.emit_constants()
            self.emit_embedding()
            for l in range(self.cfg.L):
                for m in range(self.cfg.NM):
                    self.emit_attn_pass(l, m)
                self.emit_mlp(l)
                if l == 0:
                    self.tap("xl0", self.x[:])
            self.tap("xf", self.x[:])
            self.emit_head()
        self.nc.finalize()  # bacc: register allocation + codegen passes
        return self.nc


# ---------------- host-side packing ----------------
def _bf(a):
    return np.asarray(a, dtype=np.float32).astype(ml_dtypes.bfloat16)


def _r3(w, pdim=P):
    """[K, N] -> [P, K//P, N] with K = kn*P + kp."""
    K, N = w.shape
    return np.ascontiguousarray(w.reshape(K // pdim, pdim, N).transpose(1, 0, 2))


def _rcol(v):
    """[K] -> [P, K//P] (k = kn*P + kp)."""
    return np.ascontiguousarray(v.reshape(-1, P).T)


def pack_shared(cfg: Cfg, inp):
    """Everything identical across cores."""
    sh = {}
    sh["tok_emb"] = np.ascontiguousarray(np.asarray(inp["tok_emb"], np.float32))
    pos = np.asarray(inp["pos_emb"], np.float32)[0]  # [T, D]
    sh["pos_r"] = _r3(pos)  # [P, TN, D]
    m = np.asarray(inp["masks"], np.float32)
    mT = m.transpose(0, 2, 1)  # [NM, tk, tq]
    sh["masksT_r"] = np.ascontiguousarray(
        _bf(mT).reshape(cfg.NM, cfg.TN, P, cfg.T).transpose(0, 2, 1, 3))
    for name, key in (("wq_r", "Wq"), ("wk_r", "Wk"), ("wv_r", "Wv"),
                      ("wo_r", "Wo"), ("w1_r", "W1"), ("w2_r", "W2")):
        w = _bf(inp[key])
        sh[name] = np.ascontiguousarray(
            w.reshape(cfg.L, w.shape[1] // P, P, w.shape[2]).transpose(0, 2, 1, 3))
    for name, key in (("ln1w_r", "ln1_w"), ("ln1b_r", "ln1_b"),
                      ("ln2w_r", "ln2_w"), ("ln2b_r", "ln2_b"),
                      ("bq_r", "bq"), ("bk_r", "bk")):
        v = np.asarray(inp[key], np.float32)
        if name == "bq_r":
            # the Q evacuation computes psum*scale + bias on ACT, so the
            # bias must carry the attention scale itself
            v = v / math.sqrt(cfg.DH)
        sh[name] = np.ascontiguousarray(
            v.reshape(cfg.L, -1, P).transpose(0, 2, 1))
    sh["b1_r"] = np.ascontiguousarray(
        np.asarray(inp["b1"], np.float32).reshape(cfg.L, -1, P).transpose(0, 2, 1))
    sh["bv_row"] = np.ascontiguousarray(_bf(inp["bv"])[:, None, :])
    sh["bo_row"] = np.ascontiguousarray(_bf(inp["bo"])[:, None, :])
    sh["b2_row"] = np.ascontiguousarray(_bf(inp["b2"])[:, None, :])
    sh["lnfw_r"] = _rcol(np.asarray(inp["lnf_w"], np.float32))
    sh["lnfb_r"] = _rcol(np.asarray(inp["lnf_b"], np.float32))
    return sh


def pack_core(cfg: Cfg, inp, sh, b, half, head_halves):
    m = dict(sh)
    idx = np.asarray(inp["idx"]).astype(np.int32)[b]  # [T]
    m["idx"] = np.ascontiguousarray(idx[:, None])
    m["head_r"] = head_halves[half]
    return m


def prepare(inputs, cfg=None):
    """Build the SPMD program and the 8 per-core input maps."""
    if cfg is None:
        nz = tuple(k for k in ("bv", "bo", "b2")
                   if np.any(np.asarray(inputs[k])))
        cfg = Cfg(nz=nz)
    nc = GPTBuilder(cfg).build()
    sh = pack_shared(cfg, inputs)
    hw = np.asarray(inputs["head_w"], np.float32)
    hpad = np.zeros((cfg.D, 2 * cfg.VS), np.float32)
    hpad[:, :cfg.V] = hw
    head_halves = [
        np.ascontiguousarray(_r3(_bf(hpad[:, i * cfg.VS:(i + 1) * cfg.VS])))
        for i in range(2)
    ]
    in_maps = [pack_core(cfg, inputs, sh, c // 2, c % 2, head_halves)
               for c in range(N_CORES)]
    return nc, in_maps


def assemble(cfg, results):
    logits = np.empty((B_FULL, cfg.T, cfg.V), np.float32)
    for b in range(B_FULL):
        lo = results[2 * b]["out"]
        hi = results[2 * b + 1]["out"]
        full = np.concatenate([lo, hi], axis=1)
        logits[b] = full[:, :cfg.V]
    return logits


def kernel(**inputs) -> np.ndarray:
    from concourse.bass_utils import run_bass_kernel_spmd

    cfg = Cfg()
    nc, in_maps = prepare(inputs, cfg)
    res = run_bass_kernel_spmd(nc, in_maps, list(range(N_CORES)))
    return assemble(cfg, res.results)
